# revision 4
# baseline (speedup 1.0000x reference)
"""Causal self-attention (RoPE) Trainium2 kernel.

Distribution: 8 cores = 2 data-parallel groups (batch dim, B=2) x 4
tensor-parallel cores (16 heads -> 4 heads/core).  Each core computes
QKV projection + RoPE + causal attention + output projection for its
batch and heads; a ReduceScatter over each 4-core group sums the
partial output projections and leaves each core with a 512-row shard,
which the host reassembles.

Self-contained: hardcodes all shapes from the problem spec.
"""

import numpy as np

B, T, C = 2, 2048, 2048
H, D = 16, 128
HL = 4            # heads per core
W_LOC = HL * D    # 512 local head width
NCORES = 8
GROUPS = [[0, 1, 2, 3], [4, 5, 6, 7]]
SCALE = 1.0 / float(np.sqrt(D))

_CACHE = {}


def _host_tables():
    # Mirror reference _rope_tables in float32.
    inv_freq = (1.0 / (10000.0 ** (np.arange(0, D, 2, dtype=np.float32) / np.float32(D)))).astype(np.float32)
    t = np.arange(T, dtype=np.float32)
    freqs = np.outer(t, inv_freq).astype(np.float32)        # (T, D/2)
    emb = np.concatenate([freqs, freqs], axis=-1)           # (T, D)
    cos_t = np.ascontiguousarray(np.cos(emb).astype(np.float32).T)  # (D, T)
    sin_t = np.ascontiguousarray(np.sin(emb).astype(np.float32).T)
    return cos_t, sin_t


def _host_masks():
    import ml_dtypes
    # S^T-layout causal masks for the 4 diagonal phases.
    # mask[p][kk, qq] = 1 if qq >= kk + p*128 else 0
    kk = np.arange(128)[:, None]
    qq = np.arange(512)[None, :]
    m = np.stack([(qq >= kk + p * 128) for p in range(4)]).astype(np.float32)
    return m.astype(ml_dtypes.bfloat16)


def _host_rmat():
    # rot_half(q) * sin == R @ (sin * q) in (D, t) layout, because the
    # rope table halves are identical.  R[d, d+64] = -1 (d<64),
    # R[d, d-64] = +1 (d>=64).  matmul computes lhsT.T @ rhs, so pass R^T.
    R = np.zeros((D, D), dtype=np.float32)
    for d in range(64):
        R[d, d + 64] = -1.0
        R[d + 64, d] = 1.0
    return np.ascontiguousarray(R.T)


def _build():
    if "nc" in _CACHE:
        return _CACHE["nc"]

    import concourse.mybir as mybir
    import concourse.tile as tile
    from concourse import bacc

    f32 = mybir.dt.float32
    f32r = mybir.dt.float32r
    bf16 = mybir.dt.bfloat16

    nc = bacc.Bacc(None, target_bir_lowering=False, num_devices=NCORES)

    xT = nc.dram_tensor("xT", [C, T], f32r, kind="ExternalInput")
    wq = nc.dram_tensor("wq", [C, W_LOC], f32r, kind="ExternalInput")
    wk = nc.dram_tensor("wk", [C, W_LOC], f32r, kind="ExternalInput")
    wv = nc.dram_tensor("wv", [C, W_LOC], f32r, kind="ExternalInput")
    wp = nc.dram_tensor("wp", [W_LOC, C], f32r, kind="ExternalInput")
    cos_t = nc.dram_tensor("cos_t", [D, T], f32, kind="ExternalInput")
    sin_t = nc.dram_tensor("sin_t", [D, T], f32, kind="ExternalInput")
    masks = nc.dram_tensor("masks", [4, 128, 512], bf16, kind="ExternalInput")
    rmat = nc.dram_tensor("rmat", [D, D], f32r, kind="ExternalInput")
    ident_in = nc.dram_tensor("ident", [128, 128], f32r, kind="ExternalInput")
    out_ext = nc.dram_tensor("out_shard", [T // 4, C], f32, kind="ExternalOutput")

    NCC = C // 128   # 16 contraction chunks
    HALF = T // 2    # 1024

    def bc(ap):
        return ap.bitcast(f32r)

    with tile.TileContext(nc) as tc:
        with (
            tc.tile_pool(name="const", bufs=1) as constp,
            tc.tile_pool(name="persist", bufs=1) as pers,
            tc.tile_pool(name="dram", bufs=1, space="DRAM") as dram,
        ):
            partial = dram.tile([T, C], f32)
            rs_out = dram.tile([T // 4, C], f32)

            cos_sb = constp.tile([D, T], f32)
            nc.sync.dma_start(out=cos_sb, in_=cos_t[:, :])
            sin_sb = constp.tile([D, T], f32)
            nc.sync.dma_start(out=sin_sb, in_=sin_t[:, :])
            mask_sb = []
            for p in range(4):
                mt = constp.tile([128, 512], bf16, name=f"mask{p}")
                nc.sync.dma_start(out=mt, in_=masks[p, :, :])
                mask_sb.append(mt)
            rmat_sb = constp.tile([D, D], f32r)
            nc.sync.dma_start(out=rmat_sb, in_=rmat[:, :])
            ident = constp.tile([128, 128], f32r)
            nc.sync.dma_start(out=ident, in_=ident_in[:, :])

            # Persistent activations
            qk_t = {}
            for s in range(2 * HL):   # 0-3 q heads, 4-7 k heads
                qk_t[s] = pers.tile([D, T], f32r, name=f"qk{s}")
            v_nat = []
            for tt in range(T // 128):
                vt = pers.tile([128, HL, D + 1], bf16, name=f"vnat{tt}")
                nc.vector.memset(vt[:, :, D:D + 1], 1.0)
                v_nat.append(vt)

            # ---------------- Phase 1: QKV + RoPE ----------------
            with (
                tc.tile_pool(name="xtp", bufs=1) as xtp,
                tc.tile_pool(name="wtp", bufs=1) as wtp,
                tc.tile_pool(name="ropet", bufs=2) as ropet,
                tc.tile_pool(name="psmm", bufs=3, space="PSUM") as psmm,
                tc.tile_pool(name="pstr", bufs=2, space="PSUM") as pstr,
            ):
                for th in range(2):
                    toff = th * HALF
                    xt = []
                    for cchunk in range(NCC):
                        x_tile = xtp.tile([128, HALF], f32r, tag=f"xt{cchunk}", name=f"xt{cchunk}")
                        nc.sync.dma_start(
                            out=x_tile,
                            in_=xT[cchunk * 128:(cchunk + 1) * 128, toff:toff + HALF],
                        )
                        xt.append(x_tile)

                    # q/k/v streams: head-dim on partitions (weights stationary)
                    for s in range(3 * HL):
                        wsrc = (wq, wk, wv)[s // HL]
                        h = s % HL
                        wtiles = []
                        for cchunk in range(NCC):
                            w_tile = wtp.tile([128, 128], f32r, tag=f"w{cchunk}", name=f"w{s}_{cchunk}")
                            nc.sync.dma_start(
                                out=w_tile,
                                in_=wsrc[cchunk * 128:(cchunk + 1) * 128, h * 128:(h + 1) * 128],
                            )
                            wtiles.append(w_tile)
                        for tb in range(2):
                            ps = psmm.tile([128, 512], f32, tag="mm", name=f"qkacc{s}_{tb}")
                            for cchunk in range(NCC):
                                nc.tensor.matmul(
                                    ps,
                                    lhsT=wtiles[cchunk],
                                    rhs=xt[cchunk][:, tb * 512:(tb + 1) * 512],
                                    start=(cchunk == 0),
                                    stop=(cchunk == NCC - 1),
                                )
                            tg = toff + tb * 512
                            if s < 2 * HL:
                                t1 = ropet.tile([128, 512], f32r, tag="t1", name=f"t1_{s}_{tb}")
                                nc.vector.tensor_mul(t1, ps, sin_sb[:, tg:tg + 512])
                                pr = psmm.tile([128, 512], f32, tag="mm", name=f"rot{s}_{tb}")
                                nc.tensor.matmul(pr, lhsT=rmat_sb, rhs=t1)
                                t2 = ropet.tile([128, 512], f32, tag="t2", name=f"t2_{s}_{tb}")
                                nc.vector.tensor_mul(t2, ps, cos_sb[:, tg:tg + 512])
                                nc.vector.tensor_add(qk_t[s][:, tg:tg + 512], t2, pr)
                            else:
                                # vT tile -> transpose to natural layout per 128-col block
                                vtmp = ropet.tile([128, 512], f32r, tag="t1", name=f"vtmp{s}_{tb}")
                                nc.scalar.copy(vtmp, ps)
                                for i in range(4):
                                    tt = (tg + i * 128) // 128
                                    ptr = pstr.tile([128, 128], f32, tag="tr", name=f"vtr{s}_{tb}_{i}")
                                    nc.tensor.matmul(
                                        ptr.bitcast(f32r),
                                        lhsT=vtmp[:, i * 128:(i + 1) * 128],
                                        rhs=ident,
                                        is_transpose=True,
                                    )
                                    nc.scalar.copy(v_nat[tt][:, h, 0:D], ptr)

            # ---------------- Phase 2: attention ----------------
            with (
                tc.tile_pool(name="wpp", bufs=1) as wpp,
                tc.tile_pool(name="ytp", bufs=1) as ytp,
            ):
                wp_sb = []
                for h in range(HL):
                    wp_tile = wpp.tile([128, C], f32r, name=f"wp{h}")
                    nc.sync.dma_start(out=wp_tile, in_=wp[h * 128:(h + 1) * 128, :])
                    wp_sb.append(wp_tile)
                yt_sb = []
                for h in range(HL):
                    yt_sb.append(ytp.tile([D, T], f32r, name=f"yt{h}"))

                with (
                    tc.tile_pool(name="ptp", bufs=4) as ptp,
                    tc.tile_pool(name="ynp", bufs=3) as ynp,
                    tc.tile_pool(name="pss", bufs=2, space="PSUM") as pss_p,
                    tc.tile_pool(name="psy", bufs=5, space="PSUM") as psy_p,
                    tc.tile_pool(name="pst", bufs=1, space="PSUM") as pst_p,
                ):
                    for h in range(HL):
                        for qb in range(4):          # 512-wide query blocks
                            q0 = qb * 512
                            nkb = 4 * (qb + 1)
                            psy = []
                            for qs in range(4):
                                psy.append(psy_p.tile([128, D + 1], f32, tag="y", name=f"y{h}_{qb}_{qs}"))
                            for kb in range(nkb):
                                ps_s = pss_p.tile([128, 512], f32, tag="s", name=f"s{h}_{qb}_{kb}")
                                nc.tensor.matmul(
                                    ps_s,
                                    lhsT=qk_t[HL + h][:, kb * 128:(kb + 1) * 128],
                                    rhs=qk_t[h][:, q0:q0 + 512],
                                )
                                pt = ptp.tile([128, 512], bf16, tag="pt", name=f"pt{h}_{qb}_{kb}")
                                nc.scalar.activation(
                                    pt, ps_s, mybir.ActivationFunctionType.Exp,
                                    scale=SCALE,
                                )
                                pidx = kb - 4 * qb
                                if pidx >= 0:
                                    nc.vector.tensor_mul(pt, pt, mask_sb[pidx])
                                for qs in range(4):
                                    last = 4 * qb + qs
                                    if kb <= last:
                                        nc.tensor.matmul(
                                            psy[qs],
                                            lhsT=pt[:, qs * 128:(qs + 1) * 128],
                                            rhs=v_nat[kb][:, h, :],
                                            start=(kb == 0),
                                            stop=(kb == last),
                                        )
                            for qs in range(4):
                                rec = ynp.tile([128, 1], f32, tag="rec", name=f"rec{h}_{qb}_{qs}")
                                nc.vector.reciprocal(rec, psy[qs][:, D:D + 1])
                                yn = ynp.tile([128, 128], f32r, tag="yn", name=f"yn{h}_{qb}_{qs}")
                                nc.vector.tensor_scalar_mul(yn, psy[qs][:, 0:D], rec)
                                ptr = pst_p.tile([128, 128], f32, tag="tr", name=f"ytr{h}_{qb}_{qs}")
                                nc.tensor.matmul(
                                    ptr.bitcast(f32r), lhsT=yn, rhs=ident,
                                    is_transpose=True,
                                )
                                nc.scalar.copy(
                                    yt_sb[h][:, q0 + qs * 128:q0 + (qs + 1) * 128], ptr
                                )

                # ---------------- Phase 3: output projection ----------------
                with (
                    tc.tile_pool(name="outp", bufs=3) as outp,
                    tc.tile_pool(name="pso", bufs=6, space="PSUM") as pso_p,
                ):
                    for qt in range(T // 128):
                        for cc in range(4):
                            po = pso_p.tile([128, 512], f32, tag="po", name=f"po{qt}_{cc}")
                            for h in range(HL):
                                nc.tensor.matmul(
                                    po,
                                    lhsT=yt_sb[h][:, qt * 128:(qt + 1) * 128],
                                    rhs=wp_sb[h][:, cc * 512:(cc + 1) * 512],
                                    start=(h == 0),
                                    stop=(h == HL - 1),
                                )
                            ot = outp.tile([128, 512], f32, tag="ot", name=f"ot{qt}_{cc}")
                            nc.scalar.copy(ot, po)
                            nc.sync.dma_start(
                                out=partial[qt * 128:(qt + 1) * 128, cc * 512:(cc + 1) * 512],
                                in_=ot,
                            )

            # ---------------- Phase 4: ReduceScatter + output ----------------
            import concourse.mybir as _mybir
            nc.gpsimd.collective_compute(
                "ReduceScatter",
                _mybir.AluOpType.add,
                replica_groups=GROUPS,
                ins=[partial.opt()],
                outs=[rs_out.opt()],
            )
            with tc.tile_pool(name="stg", bufs=2) as stg:
                for i in range(4):
                    s_tile = stg.tile([128, C], f32, tag="stg", name=f"stg{i}")
                    nc.sync.dma_start(out=s_tile, in_=rs_out[i * 128:(i + 1) * 128, :])
                    nc.sync.dma_start(out=out_ext[i * 128:(i + 1) * 128, :], in_=s_tile)

    nc.compile()
    _CACHE["nc"] = nc
    return nc


def _in_maps(x, Wqkv, Wproj):
    cos_t, sin_t = _host_tables()
    masks = _host_masks()
    rmat = _host_rmat()
    x = np.asarray(x, dtype=np.float32)
    Wqkv = np.asarray(Wqkv, dtype=np.float32)
    Wproj = np.asarray(Wproj, dtype=np.float32)
    maps = []
    for core in range(NCORES):
        b, r = divmod(core, 4)
        lo, hi = r * W_LOC, (r + 1) * W_LOC
        maps.append({
            "xT": np.ascontiguousarray(x[b].T),
            "wq": np.ascontiguousarray(Wqkv[:, lo:hi]),
            "wk": np.ascontiguousarray(Wqkv[:, C + lo:C + hi]),
            "wv": np.ascontiguousarray(Wqkv[:, 2 * C + lo:2 * C + hi]),
            "wp": np.ascontiguousarray(Wproj[lo:hi, :]),
            "cos_t": cos_t,
            "sin_t": sin_t,
            "masks": masks,
            "rmat": rmat,
            "ident": np.eye(128, dtype=np.float32),
        })
    return maps


def _run(x, Wqkv, Wproj, trace=False, tmpdir=None):
    from concourse.bass_utils import run_bass_kernel_spmd
    nc = _build()
    maps = _in_maps(x, Wqkv, Wproj)
    res = run_bass_kernel_spmd(
        nc, maps, core_ids=list(range(NCORES)), trace=trace, tmpdir=tmpdir
    )
    out = np.empty((B, T, C), dtype=np.float32)
    for core in range(NCORES):
        b, r = divmod(core, 4)
        out[b, r * 512:(r + 1) * 512, :] = res.results[core]["out_shard"]
    return out, res


def kernel(x, Wqkv, Wproj):
    out, _ = _run(x, Wqkv, Wproj)
    return out


# revision 5
# speedup vs baseline: 1.0521x; 1.0521x over previous
"""Causal self-attention (RoPE) Trainium2 kernel.

Distribution: 8 cores = 2 data-parallel groups (batch dim, B=2) x 4
tensor-parallel cores (16 heads -> 4 heads/core).  Each core computes
QKV projection + RoPE + causal attention + output projection for its
batch and heads; a ReduceScatter over each 4-core group sums the
partial output projections and leaves each core with a 512-row shard,
which the host reassembles.

Self-contained: hardcodes all shapes from the problem spec.
"""

import numpy as np

B, T, C = 2, 2048, 2048
H, D = 16, 128
HL = 4            # heads per core
W_LOC = HL * D    # 512 local head width
NCORES = 8
GROUPS = [[0, 1, 2, 3], [4, 5, 6, 7]]
SCALE = 1.0 / float(np.sqrt(D))

_CACHE = {}


def _host_tables():
    # Mirror reference _rope_tables in float32.
    inv_freq = (1.0 / (10000.0 ** (np.arange(0, D, 2, dtype=np.float32) / np.float32(D)))).astype(np.float32)
    t = np.arange(T, dtype=np.float32)
    freqs = np.outer(t, inv_freq).astype(np.float32)        # (T, D/2)
    emb = np.concatenate([freqs, freqs], axis=-1)           # (T, D)
    cos_t = np.ascontiguousarray(np.cos(emb).astype(np.float32).T)  # (D, T)
    sin_t = np.ascontiguousarray(np.sin(emb).astype(np.float32).T)
    return cos_t, sin_t


def _host_masks():
    import ml_dtypes
    # S^T-layout causal masks for the 4 diagonal phases.
    # mask[p][kk, qq] = 1 if qq >= kk + p*128 else 0
    kk = np.arange(128)[:, None]
    qq = np.arange(512)[None, :]
    m = np.stack([(qq >= kk + p * 128) for p in range(4)]).astype(np.float32)
    return m.astype(ml_dtypes.bfloat16)


def _host_rmat():
    # rot_half(q) * sin == R @ (sin * q) in (D, t) layout, because the
    # rope table halves are identical.  R[d, d+64] = -1 (d<64),
    # R[d, d-64] = +1 (d>=64).  matmul computes lhsT.T @ rhs, so pass R^T.
    R = np.zeros((D, D), dtype=np.float32)
    for d in range(64):
        R[d, d + 64] = -1.0
        R[d + 64, d] = 1.0
    return np.ascontiguousarray(R.T)


def _build():
    if "nc" in _CACHE:
        return _CACHE["nc"]

    import concourse.mybir as mybir
    import concourse.tile as tile
    from concourse import bacc

    f32 = mybir.dt.float32
    f32r = mybir.dt.float32r
    bf16 = mybir.dt.bfloat16

    nc = bacc.Bacc(None, target_bir_lowering=False, num_devices=NCORES)

    xT = nc.dram_tensor("xT", [C, T], f32r, kind="ExternalInput")
    wq = nc.dram_tensor("wq", [C, W_LOC], f32r, kind="ExternalInput")
    wk = nc.dram_tensor("wk", [C, W_LOC], f32r, kind="ExternalInput")
    wv = nc.dram_tensor("wv", [C, W_LOC], f32r, kind="ExternalInput")
    wp = nc.dram_tensor("wp", [W_LOC, C], f32r, kind="ExternalInput")
    cos_t = nc.dram_tensor("cos_t", [D, T], f32, kind="ExternalInput")
    sin_t = nc.dram_tensor("sin_t", [D, T], f32, kind="ExternalInput")
    masks = nc.dram_tensor("masks", [4, 128, 512], bf16, kind="ExternalInput")
    rmat = nc.dram_tensor("rmat", [D, D], f32r, kind="ExternalInput")
    ident_in = nc.dram_tensor("ident", [128, 128], f32r, kind="ExternalInput")
    out_ext = nc.dram_tensor("out_shard", [T // 4, C], f32, kind="ExternalOutput")

    NCC = C // 128   # 16 contraction chunks
    HALF = T // 2    # 1024

    def bc(ap):
        return ap.bitcast(f32r)

    with tile.TileContext(nc) as tc:
        with (
            tc.tile_pool(name="const", bufs=1) as constp,
            tc.tile_pool(name="persist", bufs=1) as pers,
            tc.tile_pool(name="dram", bufs=1, space="DRAM") as dram,
        ):
            partials = [dram.tile([512, C], f32, name=f"partial{j}") for j in range(4)]
            rs_outs = [dram.tile([128, C], f32, name=f"rsout{j}") for j in range(4)]

            cos_sb = constp.tile([D, T], f32)
            nc.sync.dma_start(out=cos_sb, in_=cos_t[:, :])
            sin_sb = constp.tile([D, T], f32)
            nc.sync.dma_start(out=sin_sb, in_=sin_t[:, :])
            mask_sb = []
            for p in range(4):
                mt = constp.tile([128, 512], bf16, name=f"mask{p}")
                nc.sync.dma_start(out=mt, in_=masks[p, :, :])
                mask_sb.append(mt)
            rmat_sb = constp.tile([D, D], f32r)
            nc.sync.dma_start(out=rmat_sb, in_=rmat[:, :])
            ident = constp.tile([128, 128], f32r)
            nc.sync.dma_start(out=ident, in_=ident_in[:, :])

            # Persistent activations
            qk_t = {}
            for s in range(2 * HL):   # 0-3 q heads, 4-7 k heads
                qk_t[s] = pers.tile([D, T], f32r, name=f"qk{s}")
            v_nat = []
            for tt in range(T // 128):
                vt = pers.tile([128, HL, D + 1], bf16, name=f"vnat{tt}")
                nc.vector.memset(vt[:, :, D:D + 1], 1.0)
                v_nat.append(vt)

            # ---------------- Phase 1: QKV + RoPE ----------------
            with (
                tc.tile_pool(name="xtp", bufs=1) as xtp,
                tc.tile_pool(name="wtp", bufs=1) as wtp,
                tc.tile_pool(name="ropet", bufs=2) as ropet,
                tc.tile_pool(name="psmm", bufs=3, space="PSUM") as psmm,
                tc.tile_pool(name="pstr", bufs=2, space="PSUM") as pstr,
            ):
                for th in range(2):
                    toff = th * HALF
                    xt = []
                    for cchunk in range(NCC):
                        x_tile = xtp.tile([128, HALF], f32r, tag=f"xt{cchunk}", name=f"xt{cchunk}")
                        nc.sync.dma_start(
                            out=x_tile,
                            in_=xT[cchunk * 128:(cchunk + 1) * 128, toff:toff + HALF],
                        )
                        xt.append(x_tile)

                    # q/k/v streams: head-dim on partitions (weights stationary)
                    for s in range(3 * HL):
                        wsrc = (wq, wk, wv)[s // HL]
                        h = s % HL
                        wtiles = []
                        for cchunk in range(NCC):
                            w_tile = wtp.tile([128, 128], f32r, tag=f"w{cchunk}", name=f"w{s}_{cchunk}")
                            nc.sync.dma_start(
                                out=w_tile,
                                in_=wsrc[cchunk * 128:(cchunk + 1) * 128, h * 128:(h + 1) * 128],
                            )
                            wtiles.append(w_tile)
                        for tb in range(2):
                            ps = psmm.tile([128, 512], f32, tag="mm", name=f"qkacc{s}_{tb}")
                            for cchunk in range(NCC):
                                nc.tensor.matmul(
                                    ps,
                                    lhsT=wtiles[cchunk],
                                    rhs=xt[cchunk][:, tb * 512:(tb + 1) * 512],
                                    start=(cchunk == 0),
                                    stop=(cchunk == NCC - 1),
                                )
                            tg = toff + tb * 512
                            if s < 2 * HL:
                                t1 = ropet.tile([128, 512], f32r, tag="t1", name=f"t1_{s}_{tb}")
                                nc.vector.tensor_mul(t1, ps, sin_sb[:, tg:tg + 512])
                                pr = psmm.tile([128, 512], f32, tag="mm", name=f"rot{s}_{tb}")
                                nc.tensor.matmul(pr, lhsT=rmat_sb, rhs=t1)
                                t2 = ropet.tile([128, 512], f32, tag="t2", name=f"t2_{s}_{tb}")
                                nc.vector.tensor_mul(t2, ps, cos_sb[:, tg:tg + 512])
                                nc.vector.tensor_add(qk_t[s][:, tg:tg + 512], t2, pr)
                            else:
                                # vT tile -> transpose to natural layout per 128-col block
                                vtmp = ropet.tile([128, 512], f32r, tag="t1", name=f"vtmp{s}_{tb}")
                                nc.scalar.copy(vtmp, ps)
                                for i in range(4):
                                    tt = (tg + i * 128) // 128
                                    ptr = pstr.tile([128, 128], f32, tag="tr", name=f"vtr{s}_{tb}_{i}")
                                    nc.tensor.matmul(
                                        ptr.bitcast(f32r),
                                        lhsT=vtmp[:, i * 128:(i + 1) * 128],
                                        rhs=ident,
                                        is_transpose=True,
                                    )
                                    nc.scalar.copy(v_nat[tt][:, h, 0:D], ptr)

            # ---------------- Phase 2: attention ----------------
            with (
                tc.tile_pool(name="wpp", bufs=1) as wpp,
                tc.tile_pool(name="ytp", bufs=1) as ytp,
            ):
                wp_sb = []
                for h in range(HL):
                    wp_tile = wpp.tile([128, C], f32r, name=f"wp{h}")
                    nc.sync.dma_start(out=wp_tile, in_=wp[h * 128:(h + 1) * 128, :])
                    wp_sb.append(wp_tile)
                yt_sb = []
                for h in range(HL):
                    yt_sb.append(ytp.tile([D, T], f32r, name=f"yt{h}"))

                with (
                    tc.tile_pool(name="ptp", bufs=4) as ptp,
                    tc.tile_pool(name="ynp", bufs=3) as ynp,
                    tc.tile_pool(name="pss", bufs=2, space="PSUM") as pss_p,
                    tc.tile_pool(name="psy", bufs=5, space="PSUM") as psy_p,
                    tc.tile_pool(name="pst", bufs=1, space="PSUM") as pst_p,
                ):
                    for h in range(HL):
                        for qb in range(4):          # 512-wide query blocks
                            q0 = qb * 512
                            nkb = 4 * (qb + 1)
                            psy = []
                            for qs in range(4):
                                psy.append(psy_p.tile([128, D + 1], f32, tag="y", name=f"y{h}_{qb}_{qs}"))
                            for kb in range(nkb):
                                ps_s = pss_p.tile([128, 512], f32, tag="s", name=f"s{h}_{qb}_{kb}")
                                nc.tensor.matmul(
                                    ps_s,
                                    lhsT=qk_t[HL + h][:, kb * 128:(kb + 1) * 128],
                                    rhs=qk_t[h][:, q0:q0 + 512],
                                )
                                pt = ptp.tile([128, 512], bf16, tag="pt", name=f"pt{h}_{qb}_{kb}")
                                nc.scalar.activation(
                                    pt, ps_s, mybir.ActivationFunctionType.Exp,
                                    scale=SCALE,
                                )
                                pidx = kb - 4 * qb
                                if pidx >= 0:
                                    nc.vector.tensor_mul(pt, pt, mask_sb[pidx])
                                for qs in range(4):
                                    last = 4 * qb + qs
                                    if kb <= last:
                                        nc.tensor.matmul(
                                            psy[qs],
                                            lhsT=pt[:, qs * 128:(qs + 1) * 128],
                                            rhs=v_nat[kb][:, h, :],
                                            start=(kb == 0),
                                            stop=(kb == last),
                                        )
                            for qs in range(4):
                                rec = ynp.tile([128, 1], f32, tag="rec", name=f"rec{h}_{qb}_{qs}")
                                nc.vector.reciprocal(rec, psy[qs][:, D:D + 1])
                                yn = ynp.tile([128, 128], f32r, tag="yn", name=f"yn{h}_{qb}_{qs}")
                                nc.vector.tensor_scalar_mul(yn, psy[qs][:, 0:D], rec)
                                ptr = pst_p.tile([128, 128], f32, tag="tr", name=f"ytr{h}_{qb}_{qs}")
                                nc.tensor.matmul(
                                    ptr.bitcast(f32r), lhsT=yn, rhs=ident,
                                    is_transpose=True,
                                )
                                nc.scalar.copy(
                                    yt_sb[h][:, q0 + qs * 128:q0 + (qs + 1) * 128], ptr
                                )

                # ---------------- Phase 3: output projection ----------------
                with (
                    tc.tile_pool(name="outp", bufs=3) as outp,
                    tc.tile_pool(name="pso", bufs=6, space="PSUM") as pso_p,
                ):
                    for qt in range(T // 128):
                        for cc in range(4):
                            po = pso_p.tile([128, 512], f32, tag="po", name=f"po{qt}_{cc}")
                            for h in range(HL):
                                nc.tensor.matmul(
                                    po,
                                    lhsT=yt_sb[h][:, qt * 128:(qt + 1) * 128],
                                    rhs=wp_sb[h][:, cc * 512:(cc + 1) * 512],
                                    start=(h == 0),
                                    stop=(h == HL - 1),
                                )
                            ot = outp.tile([128, 512], f32, tag="ot", name=f"ot{qt}_{cc}")
                            nc.scalar.copy(ot, po)
                            j, qr = divmod(qt, 4)
                            nc.sync.dma_start(
                                out=partials[j][qr * 128:(qr + 1) * 128, cc * 512:(cc + 1) * 512],
                                in_=ot,
                            )
                        if qt % 4 == 3:
                            j = qt // 4
                            nc.gpsimd.collective_compute(
                                "ReduceScatter",
                                mybir.AluOpType.add,
                                replica_groups=GROUPS,
                                ins=[partials[j].opt()],
                                outs=[rs_outs[j].opt()],
                            )

            # ---------------- Phase 4: ship RS shards ----------------
            with tc.tile_pool(name="stg", bufs=2) as stg:
                for j in range(4):
                    s_tile = stg.tile([128, C], f32, tag="stg", name=f"stg{j}")
                    nc.sync.dma_start(out=s_tile, in_=rs_outs[j][:, :])
                    nc.sync.dma_start(out=out_ext[j * 128:(j + 1) * 128, :], in_=s_tile)

    nc.compile()
    _CACHE["nc"] = nc
    return nc


def _in_maps(x, Wqkv, Wproj):
    cos_t, sin_t = _host_tables()
    masks = _host_masks()
    rmat = _host_rmat()
    x = np.asarray(x, dtype=np.float32)
    Wqkv = np.asarray(Wqkv, dtype=np.float32)
    Wproj = np.asarray(Wproj, dtype=np.float32)
    maps = []
    for core in range(NCORES):
        b, r = divmod(core, 4)
        lo, hi = r * W_LOC, (r + 1) * W_LOC
        maps.append({
            "xT": np.ascontiguousarray(x[b].T),
            "wq": np.ascontiguousarray(Wqkv[:, lo:hi]),
            "wk": np.ascontiguousarray(Wqkv[:, C + lo:C + hi]),
            "wv": np.ascontiguousarray(Wqkv[:, 2 * C + lo:2 * C + hi]),
            "wp": np.ascontiguousarray(Wproj[lo:hi, :]),
            "cos_t": cos_t,
            "sin_t": sin_t,
            "masks": masks,
            "rmat": rmat,
            "ident": np.eye(128, dtype=np.float32),
        })
    return maps


def _run(x, Wqkv, Wproj, trace=False, tmpdir=None):
    from concourse.bass_utils import run_bass_kernel_spmd
    nc = _build()
    maps = _in_maps(x, Wqkv, Wproj)
    res = run_bass_kernel_spmd(
        nc, maps, core_ids=list(range(NCORES)), trace=trace, tmpdir=tmpdir
    )
    out = np.empty((B, T, C), dtype=np.float32)
    for core in range(NCORES):
        b, r = divmod(core, 4)
        shard = res.results[core]["out_shard"]
        for j in range(4):
            out[b, j * 512 + r * 128:j * 512 + (r + 1) * 128, :] = shard[j * 128:(j + 1) * 128, :]
    return out, res


def kernel(x, Wqkv, Wproj):
    out, _ = _run(x, Wqkv, Wproj)
    return out


# revision 7
# speedup vs baseline: 1.1922x; 1.1332x over previous
"""Causal self-attention (RoPE) Trainium2 kernel.

Distribution: 8 cores = 2 data-parallel groups (batch dim, B=2) x 4
tensor-parallel cores (16 heads -> 4 heads/core).  Each core computes
QKV projection + RoPE + causal attention + output projection for its
batch and heads; a ReduceScatter over each 4-core group sums the
partial output projections and leaves each core with a 512-row shard,
which the host reassembles.

Self-contained: hardcodes all shapes from the problem spec.
"""

import numpy as np

B, T, C = 2, 2048, 2048
H, D = 16, 128
HL = 4            # heads per core
W_LOC = HL * D    # 512 local head width
NCORES = 8
GROUPS = [[0, 1, 2, 3], [4, 5, 6, 7]]
SCALE = 1.0 / float(np.sqrt(D))

_CACHE = {}


def _host_tables():
    # Mirror reference _rope_tables in float32.
    inv_freq = (1.0 / (10000.0 ** (np.arange(0, D, 2, dtype=np.float32) / np.float32(D)))).astype(np.float32)
    t = np.arange(T, dtype=np.float32)
    freqs = np.outer(t, inv_freq).astype(np.float32)        # (T, D/2)
    emb = np.concatenate([freqs, freqs], axis=-1)           # (T, D)
    cos_t = np.ascontiguousarray(np.cos(emb).astype(np.float32).T)  # (D, T)
    sin_t = np.ascontiguousarray(np.sin(emb).astype(np.float32).T)
    return cos_t, sin_t


def _host_masks():
    import ml_dtypes
    # S^T-layout causal masks for the 4 diagonal phases.
    # mask[p][kk, qq] = 1 if qq >= kk + p*128 else 0
    kk = np.arange(128)[:, None]
    qq = np.arange(512)[None, :]
    m = np.stack([(qq >= kk + p * 128) for p in range(4)]).astype(np.float32)
    return m.astype(ml_dtypes.bfloat16)


def _host_rmat():
    # rot_half(q) * sin == R @ (sin * q) in (D, t) layout, because the
    # rope table halves are identical.  R[d, d+64] = -1 (d<64),
    # R[d, d-64] = +1 (d>=64).  matmul computes lhsT.T @ rhs, so pass R^T.
    R = np.zeros((D, D), dtype=np.float32)
    for d in range(64):
        R[d, d + 64] = -1.0
        R[d + 64, d] = 1.0
    return np.ascontiguousarray(R.T)


def _build():
    if "nc" in _CACHE:
        return _CACHE["nc"]

    import concourse.mybir as mybir
    import concourse.tile as tile
    from concourse import bacc

    f32 = mybir.dt.float32
    f32r = mybir.dt.float32r
    bf16 = mybir.dt.bfloat16

    nc = bacc.Bacc(None, target_bir_lowering=False, num_devices=NCORES)

    xT = nc.dram_tensor("xT", [C, T], bf16, kind="ExternalInput")
    wq = nc.dram_tensor("wq", [C, W_LOC], bf16, kind="ExternalInput")
    wk = nc.dram_tensor("wk", [C, W_LOC], bf16, kind="ExternalInput")
    wv = nc.dram_tensor("wv", [C, W_LOC], bf16, kind="ExternalInput")
    wp = nc.dram_tensor("wp", [W_LOC, C], f32r, kind="ExternalInput")
    cos_t = nc.dram_tensor("cos_t", [D, T], f32, kind="ExternalInput")
    sin_t = nc.dram_tensor("sin_t", [D, T], f32, kind="ExternalInput")
    masks = nc.dram_tensor("masks", [4, 128, 512], bf16, kind="ExternalInput")
    rmat = nc.dram_tensor("rmat", [D, D], bf16, kind="ExternalInput")
    ident_in = nc.dram_tensor("ident", [128, 128], f32r, kind="ExternalInput")
    identb_in = nc.dram_tensor("identb", [128, 128], bf16, kind="ExternalInput")
    out_ext = nc.dram_tensor("out_shard", [T // 4, C], f32, kind="ExternalOutput")

    NCC = C // 128   # 16 contraction chunks
    HALF = T // 2    # 1024

    def bc(ap):
        return ap.bitcast(f32r)

    with tile.TileContext(nc) as tc:
        with (
            tc.tile_pool(name="const", bufs=1) as constp,
            tc.tile_pool(name="persist", bufs=1) as pers,
            tc.tile_pool(name="dram", bufs=1, space="DRAM") as dram,
        ):
            partials = [dram.tile([512, C], f32, name=f"partial{j}") for j in range(4)]
            rs_outs = [dram.tile([128, C], f32, name=f"rsout{j}") for j in range(4)]

            cos_sb = constp.tile([D, T], f32)
            nc.sync.dma_start(out=cos_sb, in_=cos_t[:, :])
            sin_sb = constp.tile([D, T], f32)
            nc.sync.dma_start(out=sin_sb, in_=sin_t[:, :])
            mask_sb = []
            for p in range(4):
                mt = constp.tile([128, 512], bf16, name=f"mask{p}")
                nc.sync.dma_start(out=mt, in_=masks[p, :, :])
                mask_sb.append(mt)
            rmat_sb = constp.tile([D, D], bf16)
            nc.sync.dma_start(out=rmat_sb, in_=rmat[:, :])
            ident = constp.tile([128, 128], f32r)
            nc.sync.dma_start(out=ident, in_=ident_in[:, :])
            identb = constp.tile([128, 128], bf16)
            nc.sync.dma_start(out=identb, in_=identb_in[:, :])

            # Persistent activations
            qk_t = {}
            for s in range(2 * HL):   # 0-3 q heads, 4-7 k heads
                qk_t[s] = pers.tile([D, T], f32r, name=f"qk{s}")
            v_nat = []
            for tt in range(T // 128):
                vt = pers.tile([128, HL, D + 1], bf16, name=f"vnat{tt}")
                nc.vector.memset(vt[:, :, D:D + 1], 1.0)
                v_nat.append(vt)

            # ---------------- Phase 1: QKV + RoPE ----------------
            with (
                tc.tile_pool(name="xtp", bufs=2) as xtp,
                tc.tile_pool(name="wtp", bufs=2) as wtp,
                tc.tile_pool(name="ropet", bufs=2) as ropet,
                tc.tile_pool(name="psmm", bufs=3, space="PSUM") as psmm,
                tc.tile_pool(name="pstr", bufs=2, space="PSUM") as pstr,
            ):
                for th in range(2):
                    toff = th * HALF
                    xt = []
                    for cchunk in range(NCC):
                        x_tile = xtp.tile([128, HALF], bf16, tag=f"xt{cchunk}", name=f"xt{cchunk}")
                        nc.sync.dma_start(
                            out=x_tile,
                            in_=xT[cchunk * 128:(cchunk + 1) * 128, toff:toff + HALF],
                        )
                        xt.append(x_tile)

                    # q/k/v streams: head-dim on partitions (weights stationary)
                    for s in range(3 * HL):
                        wsrc = (wq, wk, wv)[s // HL]
                        h = s % HL
                        wtiles = []
                        for cchunk in range(NCC):
                            w_tile = wtp.tile([128, 128], bf16, tag=f"w{cchunk}", name=f"w{s}_{cchunk}")
                            nc.sync.dma_start(
                                out=w_tile,
                                in_=wsrc[cchunk * 128:(cchunk + 1) * 128, h * 128:(h + 1) * 128],
                            )
                            wtiles.append(w_tile)
                        for tb in range(2):
                            ps = psmm.tile([128, 512], f32, tag="mm", name=f"qkacc{s}_{tb}")
                            for cchunk in range(NCC):
                                nc.tensor.matmul(
                                    ps,
                                    lhsT=wtiles[cchunk],
                                    rhs=xt[cchunk][:, tb * 512:(tb + 1) * 512],
                                    start=(cchunk == 0),
                                    stop=(cchunk == NCC - 1),
                                )
                            tg = toff + tb * 512
                            if s < 2 * HL:
                                t1 = ropet.tile([128, 512], bf16, tag="t1", name=f"t1_{s}_{tb}")
                                nc.vector.tensor_mul(t1, ps, sin_sb[:, tg:tg + 512])
                                pr = psmm.tile([128, 512], f32, tag="mm", name=f"rot{s}_{tb}")
                                nc.tensor.matmul(pr, lhsT=rmat_sb, rhs=t1)
                                t2 = ropet.tile([128, 512], f32, tag="t2", name=f"t2_{s}_{tb}")
                                nc.vector.tensor_mul(t2, ps, cos_sb[:, tg:tg + 512])
                                nc.vector.tensor_add(qk_t[s][:, tg:tg + 512], t2, pr)
                            else:
                                # vT tile -> transpose to natural layout per 128-col block
                                vtmp = ropet.tile([128, 512], f32r, tag="vt", name=f"vtmp{s}_{tb}")
                                nc.scalar.copy(vtmp, ps)
                                for i in range(4):
                                    tt = (tg + i * 128) // 128
                                    ptr = pstr.tile([128, 128], f32, tag="tr", name=f"vtr{s}_{tb}_{i}")
                                    nc.tensor.matmul(
                                        ptr.bitcast(f32r),
                                        lhsT=vtmp[:, i * 128:(i + 1) * 128],
                                        rhs=ident,
                                        is_transpose=True,
                                    )
                                    nc.scalar.copy(v_nat[tt][:, h, 0:D], ptr)

            # ---------------- Phase 2: attention ----------------
            with (
                tc.tile_pool(name="wpp", bufs=1) as wpp,
                tc.tile_pool(name="ytp", bufs=1) as ytp,
            ):
                wp_sb = []
                for h in range(HL):
                    wp_tile = wpp.tile([128, C], f32r, name=f"wp{h}")
                    nc.sync.dma_start(out=wp_tile, in_=wp[h * 128:(h + 1) * 128, :])
                    wp_sb.append(wp_tile)
                yt_sb = []
                for h in range(HL):
                    yt_sb.append(ytp.tile([D, T], f32r, name=f"yt{h}"))

                with (
                    tc.tile_pool(name="ptp", bufs=4) as ptp,
                    tc.tile_pool(name="ynp", bufs=3) as ynp,
                    tc.tile_pool(name="pss", bufs=2, space="PSUM") as pss_p,
                    tc.tile_pool(name="psy", bufs=5, space="PSUM") as psy_p,
                    tc.tile_pool(name="pst", bufs=1, space="PSUM") as pst_p,
                ):
                    for h in range(HL):
                        for qb in range(4):          # 512-wide query blocks
                            q0 = qb * 512
                            nkb = 4 * (qb + 1)
                            psy = []
                            for qs in range(4):
                                psy.append(psy_p.tile([128, D + 1], f32, tag="y", name=f"y{h}_{qb}_{qs}"))
                            for kb in range(nkb):
                                ps_s = pss_p.tile([128, 512], f32, tag="s", name=f"s{h}_{qb}_{kb}")
                                nc.tensor.matmul(
                                    ps_s,
                                    lhsT=qk_t[HL + h][:, kb * 128:(kb + 1) * 128],
                                    rhs=qk_t[h][:, q0:q0 + 512],
                                )
                                pt = ptp.tile([128, 512], bf16, tag="pt", name=f"pt{h}_{qb}_{kb}")
                                nc.scalar.activation(
                                    pt, ps_s, mybir.ActivationFunctionType.Exp,
                                    scale=SCALE,
                                )
                                pidx = kb - 4 * qb
                                if pidx >= 0:
                                    nc.vector.tensor_mul(pt, pt, mask_sb[pidx])
                                for qs in range(4):
                                    last = 4 * qb + qs
                                    if kb <= last:
                                        nc.tensor.matmul(
                                            psy[qs],
                                            lhsT=pt[:, qs * 128:(qs + 1) * 128],
                                            rhs=v_nat[kb][:, h, :],
                                            start=(kb == 0),
                                            stop=(kb == last),
                                        )
                            for qs in range(4):
                                rec = ynp.tile([128, 1], f32, tag="rec", name=f"rec{h}_{qb}_{qs}")
                                nc.vector.reciprocal(rec, psy[qs][:, D:D + 1])
                                yn = ynp.tile([128, 128], f32r, tag="yn", name=f"yn{h}_{qb}_{qs}")
                                nc.vector.tensor_scalar_mul(yn, psy[qs][:, 0:D], rec)
                                ptr = pst_p.tile([128, 128], f32, tag="tr", name=f"ytr{h}_{qb}_{qs}")
                                nc.tensor.matmul(
                                    ptr.bitcast(f32r), lhsT=yn, rhs=ident,
                                    is_transpose=True,
                                )
                                nc.vector.tensor_copy(
                                    yt_sb[h][:, q0 + qs * 128:q0 + (qs + 1) * 128], ptr
                                )

                # ---------------- Phase 3: output projection ----------------
                with (
                    tc.tile_pool(name="outp", bufs=3) as outp,
                    tc.tile_pool(name="pso", bufs=6, space="PSUM") as pso_p,
                ):
                    for qt in range(T // 128):
                        for cc in range(4):
                            po = pso_p.tile([128, 512], f32, tag="po", name=f"po{qt}_{cc}")
                            for h in range(HL):
                                nc.tensor.matmul(
                                    po,
                                    lhsT=yt_sb[h][:, qt * 128:(qt + 1) * 128],
                                    rhs=wp_sb[h][:, cc * 512:(cc + 1) * 512],
                                    start=(h == 0),
                                    stop=(h == HL - 1),
                                )
                            ot = outp.tile([128, 512], f32, tag="ot", name=f"ot{qt}_{cc}")
                            nc.scalar.copy(ot, po)
                            j, qr = divmod(qt, 4)
                            nc.sync.dma_start(
                                out=partials[j][qr * 128:(qr + 1) * 128, cc * 512:(cc + 1) * 512],
                                in_=ot,
                            )
                        if qt % 4 == 3:
                            j = qt // 4
                            nc.gpsimd.collective_compute(
                                "ReduceScatter",
                                mybir.AluOpType.add,
                                replica_groups=GROUPS,
                                ins=[partials[j].opt()],
                                outs=[rs_outs[j].opt()],
                            )

            # ---------------- Phase 4: ship RS shards ----------------
            with tc.tile_pool(name="stg", bufs=2) as stg:
                for j in range(4):
                    s_tile = stg.tile([128, C], f32, tag="stg", name=f"stg{j}")
                    nc.sync.dma_start(out=s_tile, in_=rs_outs[j][:, :])
                    nc.sync.dma_start(out=out_ext[j * 128:(j + 1) * 128, :], in_=s_tile)

    nc.compile()
    _CACHE["nc"] = nc
    return nc


def _in_maps(x, Wqkv, Wproj):
    import ml_dtypes
    bf = ml_dtypes.bfloat16
    cos_t, sin_t = _host_tables()
    masks = _host_masks()
    rmat = _host_rmat()
    x = np.asarray(x, dtype=np.float32)
    Wqkv = np.asarray(Wqkv, dtype=np.float32)
    Wproj = np.asarray(Wproj, dtype=np.float32)
    maps = []
    for core in range(NCORES):
        b, r = divmod(core, 4)
        lo, hi = r * W_LOC, (r + 1) * W_LOC
        maps.append({
            "xT": np.ascontiguousarray(x[b].T).astype(bf),
            "wq": np.ascontiguousarray(Wqkv[:, lo:hi]).astype(bf),
            "wk": np.ascontiguousarray(Wqkv[:, C + lo:C + hi]).astype(bf),
            "wv": np.ascontiguousarray(Wqkv[:, 2 * C + lo:2 * C + hi]).astype(bf),
            "wp": np.ascontiguousarray(Wproj[lo:hi, :]),
            "cos_t": cos_t,
            "sin_t": sin_t,
            "masks": masks,
            "rmat": rmat.astype(bf),
            "ident": np.eye(128, dtype=np.float32),
            "identb": np.eye(128, dtype=np.float32).astype(bf),
        })
    return maps


def _run(x, Wqkv, Wproj, trace=False, tmpdir=None):
    from concourse.bass_utils import run_bass_kernel_spmd
    nc = _build()
    maps = _in_maps(x, Wqkv, Wproj)
    res = run_bass_kernel_spmd(
        nc, maps, core_ids=list(range(NCORES)), trace=trace, tmpdir=tmpdir
    )
    out = np.empty((B, T, C), dtype=np.float32)
    for core in range(NCORES):
        b, r = divmod(core, 4)
        shard = res.results[core]["out_shard"]
        for j in range(4):
            out[b, j * 512 + r * 128:j * 512 + (r + 1) * 128, :] = shard[j * 128:(j + 1) * 128, :]
    return out, res


def kernel(x, Wqkv, Wproj):
    out, _ = _run(x, Wqkv, Wproj)
    return out


# revision 10
# speedup vs baseline: 1.2351x; 1.0360x over previous
"""Causal self-attention (RoPE) Trainium2 kernel.

Distribution: 8 cores = 2 data-parallel groups (batch dim, B=2) x 4
tensor-parallel cores (16 heads -> 4 heads/core).  Each core computes
QKV projection + RoPE + causal attention + output projection for its
batch and heads; a ReduceScatter over each 4-core group sums the
partial output projections and leaves each core with a 512-row shard,
which the host reassembles.

Self-contained: hardcodes all shapes from the problem spec.
"""

import numpy as np

B, T, C = 2, 2048, 2048
H, D = 16, 128
HL = 4            # heads per core
W_LOC = HL * D    # 512 local head width
NCORES = 8
GROUPS = [[0, 1, 2, 3], [4, 5, 6, 7]]
SCALE = 1.0 / float(np.sqrt(D))

_CACHE = {}


def _host_tables():
    # Mirror reference _rope_tables in float32.
    inv_freq = (1.0 / (10000.0 ** (np.arange(0, D, 2, dtype=np.float32) / np.float32(D)))).astype(np.float32)
    t = np.arange(T, dtype=np.float32)
    freqs = np.outer(t, inv_freq).astype(np.float32)        # (T, D/2)
    emb = np.concatenate([freqs, freqs], axis=-1)           # (T, D)
    cos_t = np.ascontiguousarray(np.cos(emb).astype(np.float32).T)  # (D, T)
    sin_t = np.ascontiguousarray(np.sin(emb).astype(np.float32).T)
    return cos_t, sin_t


def _host_masks():
    import ml_dtypes
    # S^T-layout causal masks for the 4 diagonal phases.
    # mask[p][kk, qq] = 1 if qq >= kk + p*128 else 0
    kk = np.arange(128)[:, None]
    qq = np.arange(512)[None, :]
    m = np.stack([(qq >= kk + p * 128) for p in range(4)]).astype(np.float32)
    return m.astype(ml_dtypes.bfloat16)


def _host_rmat():
    # rot_half(q) * sin == R @ (sin * q) in (D, t) layout, because the
    # rope table halves are identical.  R[d, d+64] = -1 (d<64),
    # R[d, d-64] = +1 (d>=64).  matmul computes lhsT.T @ rhs, so pass R^T.
    R = np.zeros((D, D), dtype=np.float32)
    for d in range(64):
        R[d, d + 64] = -1.0
        R[d + 64, d] = 1.0
    return np.ascontiguousarray(R.T)


def _build():
    if "nc" in _CACHE:
        return _CACHE["nc"]

    import concourse.mybir as mybir
    import concourse.tile as tile
    from concourse import bacc

    f32 = mybir.dt.float32
    f32r = mybir.dt.float32r
    bf16 = mybir.dt.bfloat16

    nc = bacc.Bacc(None, target_bir_lowering=False, num_devices=NCORES)

    xT = nc.dram_tensor("xT", [C, T], bf16, kind="ExternalInput")
    wq = nc.dram_tensor("wq", [C, W_LOC], bf16, kind="ExternalInput")
    wk = nc.dram_tensor("wk", [C, W_LOC], bf16, kind="ExternalInput")
    wv = nc.dram_tensor("wv", [C, W_LOC], bf16, kind="ExternalInput")
    wp = nc.dram_tensor("wp", [C, W_LOC], f32r, kind="ExternalInput")
    cos_t = nc.dram_tensor("cos_t", [D, T], f32, kind="ExternalInput")
    sin_t = nc.dram_tensor("sin_t", [D, T], f32, kind="ExternalInput")
    masks = nc.dram_tensor("masks", [4, 128, 512], bf16, kind="ExternalInput")
    rmat = nc.dram_tensor("rmat", [D, D], bf16, kind="ExternalInput")
    ident_in = nc.dram_tensor("ident", [128, 128], f32r, kind="ExternalInput")
    identb_in = nc.dram_tensor("identb", [128, 128], bf16, kind="ExternalInput")
    out_ext = nc.dram_tensor("out_shard", [T, W_LOC], f32, kind="ExternalOutput")

    NCC = C // 128   # 16 contraction chunks
    HALF = T // 2    # 1024

    def bc(ap):
        return ap.bitcast(f32r)

    with tile.TileContext(nc) as tc:
        with (
            tc.tile_pool(name="const", bufs=1) as constp,
            tc.tile_pool(name="persist", bufs=1) as pers,
            tc.tile_pool(name="dram", bufs=1, space="DRAM") as dram,
        ):
            yag_in = [dram.tile([512, 512], f32r, name=f"yagin{j}") for j in range(4)]
            yag_out = [dram.tile([4 * 512, 512], f32r, name=f"yagout{j}") for j in range(4)]

            cos_sb = constp.tile([D, T], f32)
            nc.sync.dma_start(out=cos_sb, in_=cos_t[:, :])
            sin_sb = constp.tile([D, T], f32)
            nc.sync.dma_start(out=sin_sb, in_=sin_t[:, :])
            mask_sb = []
            for p in range(4):
                mt = constp.tile([128, 512], bf16, name=f"mask{p}")
                nc.sync.dma_start(out=mt, in_=masks[p, :, :])
                mask_sb.append(mt)
            rmat_sb = constp.tile([D, D], bf16)
            nc.sync.dma_start(out=rmat_sb, in_=rmat[:, :])
            ident = constp.tile([128, 128], f32r)
            nc.sync.dma_start(out=ident, in_=ident_in[:, :])
            identb = constp.tile([128, 128], bf16)
            nc.sync.dma_start(out=identb, in_=identb_in[:, :])

            # Persistent activations
            qk_t = {}
            for s in range(2 * HL):   # 0-3 q heads, 4-7 k heads
                qk_t[s] = pers.tile([D, T], f32r, name=f"qk{s}")
            v_nat = []
            for tt in range(T // 128):
                vt = pers.tile([128, HL, D + 1], bf16, name=f"vnat{tt}")
                nc.vector.memset(vt[:, :, D:D + 1], 1.0)
                v_nat.append(vt)

            # ---------------- Phase 1: QKV + RoPE ----------------
            with (
                tc.tile_pool(name="xtp", bufs=2) as xtp,
                tc.tile_pool(name="wtp", bufs=2) as wtp,
                tc.tile_pool(name="ropet", bufs=2) as ropet,
                tc.tile_pool(name="psmm", bufs=3, space="PSUM") as psmm,
                tc.tile_pool(name="pstr", bufs=2, space="PSUM") as pstr,
            ):
                for th in range(2):
                    toff = th * HALF
                    xt = []
                    for cchunk in range(NCC):
                        x_tile = xtp.tile([128, HALF], bf16, tag=f"xt{cchunk}", name=f"xt{cchunk}")
                        nc.sync.dma_start(
                            out=x_tile,
                            in_=xT[cchunk * 128:(cchunk + 1) * 128, toff:toff + HALF],
                        )
                        xt.append(x_tile)

                    # q/k/v streams: head-dim on partitions (weights stationary)
                    for s in range(3 * HL):
                        wsrc = (wq, wk, wv)[s // HL]
                        h = s % HL
                        wtiles = []
                        for cchunk in range(NCC):
                            w_tile = wtp.tile([128, 128], bf16, tag=f"w{cchunk}", name=f"w{s}_{cchunk}")
                            nc.sync.dma_start(
                                out=w_tile,
                                in_=wsrc[cchunk * 128:(cchunk + 1) * 128, h * 128:(h + 1) * 128],
                            )
                            wtiles.append(w_tile)
                        for tb in range(2):
                            ps = psmm.tile([128, 512], f32, tag="mm", name=f"qkacc{s}_{tb}")
                            for cchunk in range(NCC):
                                nc.tensor.matmul(
                                    ps,
                                    lhsT=wtiles[cchunk],
                                    rhs=xt[cchunk][:, tb * 512:(tb + 1) * 512],
                                    start=(cchunk == 0),
                                    stop=(cchunk == NCC - 1),
                                )
                            tg = toff + tb * 512
                            if s < 2 * HL:
                                t1 = ropet.tile([128, 512], bf16, tag="t1", name=f"t1_{s}_{tb}")
                                nc.vector.tensor_mul(t1, ps, sin_sb[:, tg:tg + 512])
                                pr = psmm.tile([128, 512], f32, tag="mm", name=f"rot{s}_{tb}")
                                nc.tensor.matmul(pr, lhsT=rmat_sb, rhs=t1)
                                t2 = ropet.tile([128, 512], f32, tag="t2", name=f"t2_{s}_{tb}")
                                nc.vector.tensor_mul(t2, ps, cos_sb[:, tg:tg + 512])
                                nc.vector.tensor_add(qk_t[s][:, tg:tg + 512], t2, pr)
                            else:
                                # vT tile -> transpose to natural layout per 128-col block
                                vtmp = ropet.tile([128, 512], f32r, tag="vt", name=f"vtmp{s}_{tb}")
                                nc.scalar.copy(vtmp, ps)
                                for i in range(4):
                                    tt = (tg + i * 128) // 128
                                    ptr = pstr.tile([128, 128], f32, tag="tr", name=f"vtr{s}_{tb}_{i}")
                                    nc.tensor.matmul(
                                        ptr.bitcast(f32r),
                                        lhsT=vtmp[:, i * 128:(i + 1) * 128],
                                        rhs=ident,
                                        is_transpose=True,
                                    )
                                    nc.scalar.copy(v_nat[tt][:, h, 0:D], ptr)

            # ---------------- Phase 2: attention + gather + projection ----------------
            with (
                tc.tile_pool(name="wpp", bufs=1) as wpp,
                tc.tile_pool(name="ptp", bufs=4) as ptp,
                tc.tile_pool(name="ynp", bufs=3) as ynp,
                tc.tile_pool(name="ytb", bufs=3) as ytbp,
                tc.tile_pool(name="yagp", bufs=1) as yagp,
                tc.tile_pool(name="outp", bufs=2) as outp,
                tc.tile_pool(name="pss", bufs=2, space="PSUM") as pss_p,
                tc.tile_pool(name="psml", bufs=6, space="PSUM") as psml,
            ):
                wp_sb = []
                for gh in range(16):
                    wp_tile = wpp.tile([128, W_LOC], f32r, name=f"wp{gh}")
                    nc.sync.dma_start(out=wp_tile, in_=wp[gh * 128:(gh + 1) * 128, :])
                    wp_sb.append(wp_tile)

                for qb in range(4):          # 512-wide query bands
                    q0 = qb * 512
                    nkb = 4 * (qb + 1)
                    for h in range(HL):
                        psy = []
                        for qs in range(4):
                            psy.append(psml.tile([128, D + 1], f32, tag="small",
                                                 name=f"y{h}_{qb}_{qs}"))
                        for kb in range(nkb):
                            ps_s = pss_p.tile([128, 512], f32, tag="s", name=f"s{h}_{qb}_{kb}")
                            nc.tensor.matmul(
                                ps_s,
                                lhsT=qk_t[HL + h][:, kb * 128:(kb + 1) * 128],
                                rhs=qk_t[h][:, q0:q0 + 512],
                            )
                            pt = ptp.tile([128, 512], bf16, tag="pt", name=f"pt{h}_{qb}_{kb}")
                            nc.scalar.activation(
                                pt, ps_s, mybir.ActivationFunctionType.Exp,
                                scale=SCALE,
                            )
                            pidx = kb - 4 * qb
                            if pidx >= 0:
                                nc.vector.tensor_mul(pt, pt, mask_sb[pidx])
                            for qs in range(4):
                                last = 4 * qb + qs
                                if kb <= last:
                                    nc.tensor.matmul(
                                        psy[qs],
                                        lhsT=pt[:, qs * 128:(qs + 1) * 128],
                                        rhs=v_nat[kb][:, h, :],
                                        start=(kb == 0),
                                        stop=(kb == last),
                                    )
                        ytb = ytbp.tile([128, 512], f32r, tag="ytb", name=f"ytb{h}_{qb}")
                        for qs in range(4):
                            ysrc = psy[qs]
                            rec = ynp.tile([128, 1], f32, tag="rec", name=f"rec{h}_{qb}_{qs}")
                            nc.vector.reciprocal(rec, ysrc[:, D:D + 1])
                            yn = ynp.tile([128, 128], f32r, tag="yn", name=f"yn{h}_{qb}_{qs}")
                            nc.vector.tensor_scalar_mul(yn, ysrc[:, 0:D], rec)
                            ptr = psml.tile([128, 128], f32, tag="small", name=f"ytr{h}_{qb}_{qs}")
                            nc.tensor.matmul(
                                ptr.bitcast(f32r), lhsT=yn, rhs=ident,
                                is_transpose=True,
                            )
                            nc.vector.tensor_copy(ytb[:, qs * 128:(qs + 1) * 128], ptr)
                        nc.sync.dma_start(
                            out=yag_in[qb][h * 128:(h + 1) * 128, :], in_=ytb
                        )

                    nc.gpsimd.collective_compute(
                        "AllGather",
                        mybir.AluOpType.bypass,
                        replica_groups=GROUPS,
                        ins=[yag_in[qb].opt()],
                        outs=[yag_out[qb].opt()],
                    )

                    # column-sharded projection for this band
                    yag_sb = []
                    for gh in range(16):
                        yag_t = yagp.tile([128, 512], f32r, tag=f"yag{gh}",
                                          name=f"yag{qb}_{gh}")
                        nc.sync.dma_start(
                            out=yag_t, in_=yag_out[qb][gh * 128:(gh + 1) * 128, :]
                        )
                        yag_sb.append(yag_t)
                    for qc in range(4):
                        po = psml.tile([128, 512], f32, tag="small", name=f"po{qb}_{qc}")
                        for gh in range(16):
                            nc.tensor.matmul(
                                po,
                                lhsT=yag_sb[gh][:, qc * 128:(qc + 1) * 128],
                                rhs=wp_sb[gh],
                                start=(gh == 0),
                                stop=(gh == 15),
                            )
                        ot = outp.tile([128, 512], f32, tag="ot", name=f"ot{qb}_{qc}")
                        nc.scalar.copy(ot, po)
                        nc.sync.dma_start(
                            out=out_ext[q0 + qc * 128:q0 + (qc + 1) * 128, :],
                            in_=ot,
                        )

    nc.compile()
    _CACHE["nc"] = nc
    return nc


def _in_maps(x, Wqkv, Wproj):
    import ml_dtypes
    bf = ml_dtypes.bfloat16
    cos_t, sin_t = _host_tables()
    masks = _host_masks()
    rmat = _host_rmat()
    x = np.asarray(x, dtype=np.float32)
    Wqkv = np.asarray(Wqkv, dtype=np.float32)
    Wproj = np.asarray(Wproj, dtype=np.float32)
    maps = []
    for core in range(NCORES):
        b, r = divmod(core, 4)
        lo, hi = r * W_LOC, (r + 1) * W_LOC
        maps.append({
            "xT": np.ascontiguousarray(x[b].T).astype(bf),
            "wq": np.ascontiguousarray(Wqkv[:, lo:hi]).astype(bf),
            "wk": np.ascontiguousarray(Wqkv[:, C + lo:C + hi]).astype(bf),
            "wv": np.ascontiguousarray(Wqkv[:, 2 * C + lo:2 * C + hi]).astype(bf),
            "wp": np.ascontiguousarray(Wproj[:, lo:hi]),
            "cos_t": cos_t,
            "sin_t": sin_t,
            "masks": masks,
            "rmat": rmat.astype(bf),
            "ident": np.eye(128, dtype=np.float32),
            "identb": np.eye(128, dtype=np.float32).astype(bf),
        })
    return maps


def _run(x, Wqkv, Wproj, trace=False, tmpdir=None):
    from concourse.bass_utils import run_bass_kernel_spmd
    nc = _build()
    maps = _in_maps(x, Wqkv, Wproj)
    res = run_bass_kernel_spmd(
        nc, maps, core_ids=list(range(NCORES)), trace=trace, tmpdir=tmpdir
    )
    out = np.empty((B, T, C), dtype=np.float32)
    for core in range(NCORES):
        b, r = divmod(core, 4)
        out[b, :, r * W_LOC:(r + 1) * W_LOC] = res.results[core]["out_shard"]
    return out, res


def kernel(x, Wqkv, Wproj):
    out, _ = _run(x, Wqkv, Wproj)
    return out


# revision 11
# speedup vs baseline: 1.3778x; 1.1155x over previous
"""Causal self-attention (RoPE) Trainium2 kernel.

Distribution: 8 cores = 2 data-parallel groups (batch dim, B=2) x 4
tensor-parallel cores (16 heads -> 4 heads/core).  Each core computes
QKV projection + RoPE + causal attention + output projection for its
batch and heads; a ReduceScatter over each 4-core group sums the
partial output projections and leaves each core with a 512-row shard,
which the host reassembles.

Self-contained: hardcodes all shapes from the problem spec.
"""

import numpy as np

B, T, C = 2, 2048, 2048
H, D = 16, 128
HL = 4            # heads per core
W_LOC = HL * D    # 512 local head width
NCORES = 8
GROUPS = [[0, 1, 2, 3], [4, 5, 6, 7]]
SCALE = 1.0 / float(np.sqrt(D))

_CACHE = {}


def _host_tables():
    # Mirror reference _rope_tables in float32.
    inv_freq = (1.0 / (10000.0 ** (np.arange(0, D, 2, dtype=np.float32) / np.float32(D)))).astype(np.float32)
    t = np.arange(T, dtype=np.float32)
    freqs = np.outer(t, inv_freq).astype(np.float32)        # (T, D/2)
    emb = np.concatenate([freqs, freqs], axis=-1)           # (T, D)
    cos_t = np.ascontiguousarray(np.cos(emb).astype(np.float32).T)  # (D, T)
    sin_t = np.ascontiguousarray(np.sin(emb).astype(np.float32).T)
    return cos_t, sin_t


def _host_masks():
    import ml_dtypes
    # S^T-layout causal masks for the 4 diagonal phases.
    # mask[p][kk, qq] = 1 if qq >= kk + p*128 else 0
    kk = np.arange(128)[:, None]
    qq = np.arange(512)[None, :]
    m = np.stack([(qq >= kk + p * 128) for p in range(4)]).astype(np.float32)
    return m.astype(ml_dtypes.bfloat16)


def _host_rmat():
    # rot_half(q) * sin == R @ (sin * q) in (D, t) layout, because the
    # rope table halves are identical.  R[d, d+64] = -1 (d<64),
    # R[d, d-64] = +1 (d>=64).  matmul computes lhsT.T @ rhs, so pass R^T.
    R = np.zeros((D, D), dtype=np.float32)
    for d in range(64):
        R[d, d + 64] = -1.0
        R[d + 64, d] = 1.0
    return np.ascontiguousarray(R.T)


def _build():
    if "nc" in _CACHE:
        return _CACHE["nc"]

    import concourse.mybir as mybir
    import concourse.tile as tile
    from concourse import bacc

    f32 = mybir.dt.float32
    f32r = mybir.dt.float32r
    bf16 = mybir.dt.bfloat16

    nc = bacc.Bacc(None, target_bir_lowering=False, num_devices=NCORES)

    xT = nc.dram_tensor("xT", [C, T], bf16, kind="ExternalInput")
    wq = nc.dram_tensor("wq", [C, W_LOC], bf16, kind="ExternalInput")
    wk = nc.dram_tensor("wk", [C, W_LOC], bf16, kind="ExternalInput")
    wv = nc.dram_tensor("wv", [C, W_LOC], bf16, kind="ExternalInput")
    wp = nc.dram_tensor("wp", [C, W_LOC], f32r, kind="ExternalInput")
    cos_t = nc.dram_tensor("cos_t", [D, T], f32, kind="ExternalInput")
    sin_t = nc.dram_tensor("sin_t", [D, T], f32, kind="ExternalInput")
    masks = nc.dram_tensor("masks", [4, 128, 512], bf16, kind="ExternalInput")
    rmat = nc.dram_tensor("rmat", [D, D], bf16, kind="ExternalInput")
    ident_in = nc.dram_tensor("ident", [128, 128], f32r, kind="ExternalInput")
    identb_in = nc.dram_tensor("identb", [128, 128], bf16, kind="ExternalInput")
    out_ext = nc.dram_tensor("out_shard", [T, W_LOC], f32, kind="ExternalOutput")

    NCC = C // 128   # 16 contraction chunks
    HALF = T // 2    # 1024

    def bc(ap):
        return ap.bitcast(f32r)

    with tile.TileContext(nc) as tc:
        with (
            tc.tile_pool(name="const", bufs=1) as constp,
            tc.tile_pool(name="persist", bufs=1) as pers,
            tc.tile_pool(name="dram", bufs=1, space="DRAM") as dram,
        ):
            yag_in = [dram.tile([512, 512], f32r, name=f"yagin{j}") for j in range(4)]
            yag_out = [dram.tile([4 * 512, 512], f32r, name=f"yagout{j}") for j in range(4)]

            cos_sb = constp.tile([D, T], f32)
            nc.sync.dma_start(out=cos_sb, in_=cos_t[:, :])
            sin_sb = constp.tile([D, T], f32)
            nc.sync.dma_start(out=sin_sb, in_=sin_t[:, :])
            mask_sb = []
            for p in range(4):
                mt = constp.tile([128, 512], bf16, name=f"mask{p}")
                nc.sync.dma_start(out=mt, in_=masks[p, :, :])
                mask_sb.append(mt)
            rmat_sb = constp.tile([D, D], bf16)
            nc.sync.dma_start(out=rmat_sb, in_=rmat[:, :])
            ident = constp.tile([128, 128], f32r)
            nc.sync.dma_start(out=ident, in_=ident_in[:, :])
            identb = constp.tile([128, 128], bf16)
            nc.sync.dma_start(out=identb, in_=identb_in[:, :])

            # Persistent activations
            qk_t = {}
            for s in range(2 * HL):   # 0-3 q heads, 4-7 k heads
                qk_t[s] = pers.tile([D, T], f32r, name=f"qk{s}")
            v_nat = []
            for tt in range(T // 128):
                vt = pers.tile([128, HL, D + 1], bf16, name=f"vnat{tt}")
                nc.vector.memset(vt[:, :, D:D + 1], 1.0)
                v_nat.append(vt)

            # ---------------- Phase 1: QKV + RoPE ----------------
            with (
                tc.tile_pool(name="xtp", bufs=2) as xtp,
                tc.tile_pool(name="wtp", bufs=3) as wtp,
                tc.tile_pool(name="ropet", bufs=2) as ropet,
                tc.tile_pool(name="psmm", bufs=3, space="PSUM") as psmm,
                tc.tile_pool(name="pstr", bufs=2, space="PSUM") as pstr,
            ):
                for th in range(2):
                    toff = th * HALF
                    xt = []
                    for cchunk in range(NCC):
                        x_tile = xtp.tile([128, HALF], bf16, tag=f"xt{cchunk}", name=f"xt{cchunk}")
                        nc.sync.dma_start(
                            out=x_tile,
                            in_=xT[cchunk * 128:(cchunk + 1) * 128, toff:toff + HALF],
                        )
                        xt.append(x_tile)

                    # q/k/v streams: head-dim on partitions (weights stationary)
                    for s in range(3 * HL):
                        wsrc = (wq, wk, wv)[s // HL]
                        h = s % HL
                        w_sb = wtp.tile([128, NCC, 128], bf16, tag="w", name=f"w{th}_{s}")
                        nc.sync.dma_start(
                            out=w_sb,
                            in_=wsrc[:, h * 128:(h + 1) * 128].rearrange(
                                "(cc p) n -> p cc n", p=128
                            ),
                        )
                        for tb in range(2):
                            ps = psmm.tile([128, 512], f32, tag="mm", name=f"qkacc{s}_{tb}")
                            for cchunk in range(NCC):
                                nc.tensor.matmul(
                                    ps,
                                    lhsT=w_sb[:, cchunk, :],
                                    rhs=xt[cchunk][:, tb * 512:(tb + 1) * 512],
                                    start=(cchunk == 0),
                                    stop=(cchunk == NCC - 1),
                                )
                            tg = toff + tb * 512
                            if s < 2 * HL:
                                t1 = ropet.tile([128, 512], bf16, tag="t1", name=f"t1_{s}_{tb}")
                                nc.vector.tensor_mul(t1, ps, sin_sb[:, tg:tg + 512])
                                pr = psmm.tile([128, 512], f32, tag="mm", name=f"rot{s}_{tb}")
                                nc.tensor.matmul(pr, lhsT=rmat_sb, rhs=t1)
                                t2 = ropet.tile([128, 512], f32, tag="t2", name=f"t2_{s}_{tb}")
                                nc.vector.tensor_mul(t2, ps, cos_sb[:, tg:tg + 512])
                                nc.vector.tensor_add(qk_t[s][:, tg:tg + 512], t2, pr)
                            else:
                                # vT tile -> transpose to natural layout per 128-col block
                                vtmp = ropet.tile([128, 512], f32r, tag="vt", name=f"vtmp{s}_{tb}")
                                nc.scalar.copy(vtmp, ps)
                                for i in range(4):
                                    tt = (tg + i * 128) // 128
                                    ptr = pstr.tile([128, 128], f32, tag="tr", name=f"vtr{s}_{tb}_{i}")
                                    nc.tensor.matmul(
                                        ptr.bitcast(f32r),
                                        lhsT=vtmp[:, i * 128:(i + 1) * 128],
                                        rhs=ident,
                                        is_transpose=True,
                                    )
                                    nc.scalar.copy(v_nat[tt][:, h, 0:D], ptr)

            # ---------------- Phase 2: attention + gather + projection ----------------
            with (
                tc.tile_pool(name="wpp", bufs=1) as wpp,
                tc.tile_pool(name="ptp", bufs=4) as ptp,
                tc.tile_pool(name="ynp", bufs=3) as ynp,
                tc.tile_pool(name="ytb", bufs=3) as ytbp,
                tc.tile_pool(name="yagp", bufs=1) as yagp,
                tc.tile_pool(name="outp", bufs=2) as outp,
                tc.tile_pool(name="pss", bufs=2, space="PSUM") as pss_p,
                tc.tile_pool(name="psml", bufs=6, space="PSUM") as psml,
            ):
                wp_sb = []
                for gh in range(16):
                    wp_tile = wpp.tile([128, W_LOC], f32r, name=f"wp{gh}")
                    nc.sync.dma_start(out=wp_tile, in_=wp[gh * 128:(gh + 1) * 128, :])
                    wp_sb.append(wp_tile)

                def attention_band(qb):
                    q0 = qb * 512
                    nkb = 4 * (qb + 1)
                    for h in range(HL):
                        psy = []
                        for qs in range(4):
                            psy.append(psml.tile([128, D + 1], f32, tag="small",
                                                 name=f"y{h}_{qb}_{qs}"))
                        for kb in range(nkb):
                            ps_s = pss_p.tile([128, 512], f32, tag="s", name=f"s{h}_{qb}_{kb}")
                            nc.tensor.matmul(
                                ps_s,
                                lhsT=qk_t[HL + h][:, kb * 128:(kb + 1) * 128],
                                rhs=qk_t[h][:, q0:q0 + 512],
                            )
                            pt = ptp.tile([128, 512], bf16, tag="pt", name=f"pt{h}_{qb}_{kb}")
                            nc.scalar.activation(
                                pt, ps_s, mybir.ActivationFunctionType.Exp,
                                scale=SCALE,
                            )
                            pidx = kb - 4 * qb
                            if pidx >= 0:
                                nc.vector.tensor_mul(pt, pt, mask_sb[pidx])
                            for qs in range(4):
                                last = 4 * qb + qs
                                if kb <= last:
                                    nc.tensor.matmul(
                                        psy[qs],
                                        lhsT=pt[:, qs * 128:(qs + 1) * 128],
                                        rhs=v_nat[kb][:, h, :],
                                        start=(kb == 0),
                                        stop=(kb == last),
                                    )
                        ytb = ytbp.tile([128, 512], f32r, tag="ytb", name=f"ytb{h}_{qb}")
                        for qs in range(4):
                            ysrc = psy[qs]
                            rec = ynp.tile([128, 1], f32, tag="rec", name=f"rec{h}_{qb}_{qs}")
                            nc.vector.reciprocal(rec, ysrc[:, D:D + 1])
                            yn = ynp.tile([128, 128], f32r, tag="yn", name=f"yn{h}_{qb}_{qs}")
                            nc.vector.tensor_scalar_mul(yn, ysrc[:, 0:D], rec)
                            ptr = psml.tile([128, 128], f32, tag="small", name=f"ytr{h}_{qb}_{qs}")
                            nc.tensor.matmul(
                                ptr.bitcast(f32r), lhsT=yn, rhs=ident,
                                is_transpose=True,
                            )
                            nc.vector.tensor_copy(ytb[:, qs * 128:(qs + 1) * 128], ptr)
                        nc.sync.dma_start(
                            out=yag_in[qb][h * 128:(h + 1) * 128, :], in_=ytb
                        )

                    nc.gpsimd.collective_compute(
                        "AllGather",
                        mybir.AluOpType.bypass,
                        replica_groups=GROUPS,
                        ins=[yag_in[qb].opt()],
                        outs=[yag_out[qb].opt()],
                    )

                def proj_band(qb):
                    q0 = qb * 512
                    # column-sharded projection for this band
                    yag_sb = []
                    for gh in range(16):
                        yag_t = yagp.tile([128, 512], f32r, tag=f"yag{gh}",
                                          name=f"yag{qb}_{gh}")
                        nc.sync.dma_start(
                            out=yag_t, in_=yag_out[qb][gh * 128:(gh + 1) * 128, :]
                        )
                        yag_sb.append(yag_t)
                    for qc in range(4):
                        po = psml.tile([128, 512], f32, tag="small", name=f"po{qb}_{qc}")
                        for gh in range(16):
                            nc.tensor.matmul(
                                po,
                                lhsT=yag_sb[gh][:, qc * 128:(qc + 1) * 128],
                                rhs=wp_sb[gh],
                                start=(gh == 0),
                                stop=(gh == 15),
                            )
                        ot = outp.tile([128, 512], f32, tag="ot", name=f"ot{qb}_{qc}")
                        nc.scalar.copy(ot, po)
                        nc.sync.dma_start(
                            out=out_ext[q0 + qc * 128:q0 + (qc + 1) * 128, :],
                            in_=ot,
                        )

                prev = None
                for qb in (3, 2, 1, 0):
                    attention_band(qb)
                    if prev is not None:
                        proj_band(prev)
                    prev = qb
                proj_band(prev)

    nc.compile()
    _CACHE["nc"] = nc
    return nc


def _in_maps(x, Wqkv, Wproj):
    import ml_dtypes
    bf = ml_dtypes.bfloat16
    cos_t, sin_t = _host_tables()
    masks = _host_masks()
    rmat = _host_rmat()
    x = np.asarray(x, dtype=np.float32)
    Wqkv = np.asarray(Wqkv, dtype=np.float32)
    Wproj = np.asarray(Wproj, dtype=np.float32)
    maps = []
    for core in range(NCORES):
        b, r = divmod(core, 4)
        lo, hi = r * W_LOC, (r + 1) * W_LOC
        maps.append({
            "xT": np.ascontiguousarray(x[b].T).astype(bf),
            "wq": np.ascontiguousarray(Wqkv[:, lo:hi]).astype(bf),
            "wk": np.ascontiguousarray(Wqkv[:, C + lo:C + hi]).astype(bf),
            "wv": np.ascontiguousarray(Wqkv[:, 2 * C + lo:2 * C + hi]).astype(bf),
            "wp": np.ascontiguousarray(Wproj[:, lo:hi]),
            "cos_t": cos_t,
            "sin_t": sin_t,
            "masks": masks,
            "rmat": rmat.astype(bf),
            "ident": np.eye(128, dtype=np.float32),
            "identb": np.eye(128, dtype=np.float32).astype(bf),
        })
    return maps


def _run(x, Wqkv, Wproj, trace=False, tmpdir=None):
    from concourse.bass_utils import run_bass_kernel_spmd
    nc = _build()
    maps = _in_maps(x, Wqkv, Wproj)
    res = run_bass_kernel_spmd(
        nc, maps, core_ids=list(range(NCORES)), trace=trace, tmpdir=tmpdir
    )
    out = np.empty((B, T, C), dtype=np.float32)
    for core in range(NCORES):
        b, r = divmod(core, 4)
        out[b, :, r * W_LOC:(r + 1) * W_LOC] = res.results[core]["out_shard"]
    return out, res


def kernel(x, Wqkv, Wproj):
    out, _ = _run(x, Wqkv, Wproj)
    return out
